# revision 58
# baseline (speedup 1.0000x reference)
"""3-layer GAT (PPI-style) forward on 8 Trainium2 NeuronCores.

Strategy (SPMD, one NEFF on 8 cores):
  - Host: degree-banded node permutation: nodes sorted by in-degree, dealt
    round-robin into 8 cores x 20 tiles of 128 dst rows, so every tile-slot t
    has the same per-row slot count K_t on all cores (<5% slot padding).
  - dst-ALIGNED edge layout: partition p of tile t owns dst row p; its
    incoming edges occupy slots j=0..deg-1 (chunk j).  Aggregation is then
    psum[p, :] += we[p,j] * G[p, j, :] done as PE matmuls with DIAGONAL
    lhsT = diag(we[:, j, h]) -- no one-hot builds, no per-edge dst gather.
  - Self-loops excluded from the gather; handled via one contiguous DMA of
    the tile's own payload rows (pshard) as an extra chunk.
  - Attention dots es/ed folded into the dense matmul on the host:
    waug = [W | Wl | W@a_s | W@a_d]; es/ed come out as 2H extra psum cols.
  - Payload per node: h in fp8e4 (scaled 1/8) + es in f32 tail; AllGather
    payload across cores; per-edge gather of 1280B (L1/2) / 768B (L3) rows.
  - exp(leakyrelu(es+ed)) exact softmax (no max-subtraction; |t| < ~9),
    padding slots killed via -1e30 mask added to the logit.
"""

import math
import numpy as np

N_CORES = 8
FP8L = {1: False, 2: False, 3: False}  # payload h dtype per layer: fp8e4 vs bf16
STRIDE0 = True      # batched stride-0 broadcast DVE ops (diag build etc.)


# --------------------------------------------------------------------------
# host-side prep (pure data layout / graph partitioning, no model math)
# --------------------------------------------------------------------------

def _wrap16_rep(a):
    """[L] int -> [128, L/16] int16 (16-wrap, replicated 8x down partitions)."""
    w = a.reshape(-1, 16).T.astype(np.int16)
    return np.ascontiguousarray(np.tile(w, (8, 1)))


def _host_prep(inputs, n_cores=N_CORES):
    import ml_dtypes

    bf16 = ml_dtypes.bfloat16
    x = np.asarray(inputs["x"], np.float32)
    ei = np.asarray(inputs["edge_index"])
    n, f_in = x.shape
    src = ei[0].astype(np.int64)
    dst = ei[1].astype(np.int64)

    per_core = n // n_cores                      # 2500
    T = math.ceil(per_core / 128)                # 20
    rows_last = per_core - (T - 1) * 128         # 68

    # ---- degree-banded permutation (self-loops handled separately) -------
    rows_pad = T * 128
    deg = np.bincount(dst, minlength=n).astype(np.int64)
    order = np.argsort(-deg, kind="stable")
    perm = np.empty(n, np.int64)       # output-row space (per_core rows/core)
    perm_pay = np.empty(n, np.int64)   # payload-row space (rows_pad rows/core)
    K_t = []
    pos = 0
    for t in range(T):
        rows = 128 if t < T - 1 else rows_last
        band = order[pos : pos + rows * n_cores]
        pos += rows * n_cores
        K_t.append(max(1, int(deg[band].max())))
        idx = np.arange(band.shape[0])
        c = idx % n_cores
        r = idx // n_cores
        perm[band] = c * per_core + t * 128 + r
        perm_pay[band] = c * rows_pad + t * 128 + r
    K_off = np.concatenate([[0], np.cumsum(K_t)]).astype(np.int64)
    K_sum = int(K_off[-1])

    src_n = perm_pay[src]              # gather indices -> payload rows
    dst_n = perm[dst]

    # ---- per-core slot arrays -------------------------------------------
    src16_list, mask_list = [], []
    core_of = dst_n // per_core
    for c in range(n_cores):
        sel = core_of == c
        s, d = src_n[sel], dst_n[sel]
        loc = d - c * per_core
        t_of = loc // 128
        r_of = loc - t_of * 128
        flat_parts = []
        mask = np.full((128, K_sum), -300.0, np.float32)
        for t in range(T):
            K = K_t[t]
            m = t_of == t
            rr = r_of[m]
            ss = s[m]
            o = np.argsort(rr, kind="stable")
            rr, ss = rr[o], ss[o]
            flat = np.zeros(K * 128, np.int64)
            # occurrence index per row
            occ = np.zeros_like(rr)
            if rr.size:
                chg = np.concatenate([[True], rr[1:] != rr[:-1]])
                idx0 = np.flatnonzero(chg)
                occ = np.arange(rr.size) - np.repeat(idx0, np.diff(np.concatenate([idx0, [rr.size]])))
            flat[occ * 128 + rr] = ss
            mask[rr, K_off[t] + occ] = 0.0
            flat_parts.append(flat)
        src16_list.append(np.concatenate([_wrap16_rep(f) for f in flat_parts], axis=1))
        mask_list.append(np.ascontiguousarray(mask))

    # ---- permuted node features, transposed, padded rows, bf16, per core
    x_perm = np.zeros((n, f_in), np.float32)
    x_perm[perm] = x
    xT = []
    for c in range(n_cores):
        blk = np.zeros((rows_pad, f_in), np.float32)
        blk[:per_core] = x_perm[c * per_core : (c + 1) * per_core]
        xT.append(np.ascontiguousarray(blk.T).astype(bf16))

    # ---- weights: waug = [W | Wl | W@a_s | W@a_d], bias pre-summed -------
    g = lambda k: np.asarray(inputs[k], np.float32)
    rep = lambda v: np.ascontiguousarray(np.broadcast_to(v[None, :], (128, v.shape[0]))).astype(np.float32)

    def fold(Wk, Wlk, ask, adk):
        W, Wl = g(Wk), g(Wlk)
        a_s, a_d = g(ask), g(adk)
        h_, c_ = a_s.shape
        din = W.shape[0]
        vs = np.stack([W[:, i * c_ : (i + 1) * c_] @ a_s[i] for i in range(h_)], 1)
        vd = np.stack([W[:, i * c_ : (i + 1) * c_] @ a_d[i] for i in range(h_)], 1)
        return np.ascontiguousarray(np.concatenate([W, Wl, vs, vd], 1)).astype(bf16)

    waug1 = fold("W1", "Wl1", "a1s", "a1d")      # [50, 2056]
    waug2 = fold("W2", "Wl2", "a2s", "a2d")      # [1024, 2056]
    waug3 = fold("W3", "Wl3", "a3s", "a3d")      # [1024, 859]

    base = dict(
        waug1=waug1, waug2=waug2, waug3=waug3,
        bsum1=rep(g("b1") + g("bl1")),
        bsum2=rep(g("b2") + g("bl2")),
        bsum3=rep(g("b3") + g("bl3")),
    )
    in_maps = []
    for c in range(n_cores):
        m = dict(base)
        m["xT1"] = xT[c]
        m["src16"] = src16_list[c]
        m["maskneg"] = mask_list[c]
        in_maps.append(m)

    h1, c1 = np.asarray(inputs["a1s"]).shape
    h3, c3 = np.asarray(inputs["a3s"]).shape
    cfg = dict(
        n=n, f_in=f_in, n_cores=n_cores, per_core=per_core,
        T=T, rows_last=rows_last, rows_pad=rows_pad,
        K_t=K_t, K_off=[int(v) for v in K_off], K_sum=K_sum,
        h1=h1, c1=c1, d1=h1 * c1, h3=h3, c3=c3,
    )
    return in_maps, cfg, perm


# --------------------------------------------------------------------------
# bass program
# --------------------------------------------------------------------------

def _layer_dims(cfg):
    out = []
    for li in (1, 2, 3):
        if li < 3:
            h, c = cfg["h1"], cfg["c1"]
            din = cfg["f_in"] if li == 1 else cfg["d1"]
            nlin = cfg["d1"]
        else:
            h, c = cfg["h3"], cfg["c3"]
            din = cfg["d1"]
            nlin = cfg["c3"]
        hc = h * c
        nw = hc + nlin + 2 * h                 # psum cols: h | lin | es | ed
        fp8 = FP8L[li]
        psz = 1 if fp8 else 2                  # payload h dtype size
        hb = hc * psz                          # h bytes in payload
        esb = math.ceil(hb / 8) * 8            # es byte offset (8-align)
        pwb = math.ceil((esb + 4 * h) / 256) * 256   # payload bytes
        kch = math.ceil(din / 128)
        out.append(dict(li=li, din=din, kch=kch, h=h, c=c, hc=hc, nlin=nlin,
                        nw=nw, hb=hb, esf=esb // 4, pwb=pwb, fp8=fp8,
                        sc=0.125 if fp8 else 1.0))
    return out


def _build(cfg):
    import concourse.bass as bass
    import concourse.bacc as bacc
    import concourse.mybir as mybir
    import concourse.tile as tile
    from contextlib import ExitStack

    f32 = mybir.dt.float32
    bf = mybir.dt.bfloat16
    i16 = mybir.dt.int16
    i32 = mybir.dt.int32
    u8 = mybir.dt.uint8
    fp8e4 = mybir.dt.float8e4
    EXP = mybir.ActivationFunctionType.Exp
    COPY = mybir.ActivationFunctionType.Copy
    ALU = mybir.AluOpType
    AX = mybir.AxisListType.X

    n_cores = cfg["n_cores"]
    n = cfg["n"]
    T = cfg["T"]
    rows_last = cfg["rows_last"]
    per_core = cfg["per_core"]
    rows_pad = cfg["rows_pad"]
    K_t = cfg["K_t"]
    K_off = cfg["K_off"]
    K_sum = cfg["K_sum"]
    K_max = max(K_t)
    D1 = cfg["d1"]
    layers = _layer_dims(cfg)
    HMAX = max(L["h"] for L in layers)

    nc = bacc.Bacc(None, target_bir_lowering=False)

    # ---- parameters -----------------------------------------------------
    xT1 = nc.declare_dram_parameter("xT1", [cfg["f_in"], rows_pad], bf, isOutput=False)
    waug_p = {L["li"]: nc.declare_dram_parameter(f"waug{L['li']}", [L["din"], L["nw"]], bf, isOutput=False)
              for L in layers}
    bsum_p = {L["li"]: nc.declare_dram_parameter(f"bsum{L['li']}", [128, L["nlin"]], f32, isOutput=False)
              for L in layers}
    src16_p = nc.declare_dram_parameter("src16", [128, K_sum * 8], i16, isOutput=False)
    mask_p = nc.declare_dram_parameter("maskneg", [128, K_sum], f32, isOutput=False)
    out_p = nc.declare_dram_parameter("out", [per_core, cfg["c3"]], f32, isOutput=True)

    with tile.TileContext(nc, num_cores=n_cores) as tc, ExitStack() as ctx:
        # ---- dram scratch ----------------------------------------------
        dram = ctx.enter_context(tc.tile_pool(name="dram", bufs=1, space="DRAM"))
        u16 = mybir.dt.uint16
        pshard = {L["li"]: dram.tile([rows_pad, L["pwb"] // 2], u16, tag=f"pshard{L['li']}", name=f"pshard{L['li']}")
                  for L in layers}
        pfull = {L["li"]: dram.tile([n_cores * rows_pad, L["pwb"] // 2], u16, tag=f"pfull{L['li']}", name=f"pfull{L['li']}",
                                    addr_space="Shared") for L in layers}
        xrows = {li: dram.tile([rows_pad, D1], bf, tag=f"xrows{li}", name=f"xrows{li}") for li in (1, 2)}
        linb = {L["li"]: dram.tile([rows_pad, L["nlin"]], bf, tag=f"lin{L['li']}", name=f"lin{L['li']}")
                for L in layers}

        # ---- pools ------------------------------------------------------
        consts = ctx.enter_context(tc.tile_pool(name="consts", bufs=1))
        waugp = ctx.enter_context(tc.tile_pool(name="waugp", bufs=1))
        bsump = ctx.enter_context(tc.tile_pool(name="bsump", bufs=1))
        edtp = ctx.enter_context(tc.tile_pool(name="edtp", bufs=1))
        xtp = ctx.enter_context(tc.tile_pool(name="xtp", bufs=3))
        ptp = ctx.enter_context(tc.tile_pool(name="ptp", bufs=3))
        ltp = ctx.enter_context(tc.tile_pool(name="ltp", bufs=2))
        gp = ctx.enter_context(tc.tile_pool(name="gp", bufs=8))
        sgp = ctx.enter_context(tc.tile_pool(name="sgp", bufs=2))
        idxp = ctx.enter_context(tc.tile_pool(name="idxp", bufs=2))
        wep = ctx.enter_context(tc.tile_pool(name="wep", bufs=2))
        dgp = ctx.enter_context(tc.tile_pool(name="dgp", bufs=2))
        epip = ctx.enter_context(tc.tile_pool(name="epip", bufs=1))
        recp = ctx.enter_context(tc.tile_pool(name="recp", bufs=4))
        psum_d = ctx.enter_context(tc.tile_pool(name="psum_d", bufs=1, space="PSUM"))
        psum_a = ctx.enter_context(tc.tile_pool(name="psum_a", bufs=1, space="PSUM"))

        # ---- constants ---------------------------------------------------
        iota_i = consts.tile([128, 128], i32, tag="iota_i")
        nc.gpsimd.iota(iota_i[:, :], pattern=[[1, 128]], base=0, channel_multiplier=0)
        pidx_i = consts.tile([128, 1], i32, tag="pidx_i")
        nc.gpsimd.iota(pidx_i[:, :], pattern=[[1, 1]], base=0, channel_multiplier=1)
        iota_f = consts.tile([128, 128], f32, tag="iota_f")
        nc.vector.tensor_copy(iota_f[:, :], iota_i[:, :])
        pidx_f = consts.tile([128, 1], f32, tag="pidx_f")
        nc.vector.tensor_copy(pidx_f[:, :], pidx_i[:, :])
        ident = consts.tile([128, 128], bf, tag="ident")
        nc.vector.tensor_scalar(out=ident[:, :], in0=iota_f[:, :],
                                scalar1=pidx_f[:, 0:1], scalar2=None, op0=ALU.is_equal)

        def rows_of(t):
            return 128 if t < T - 1 else rows_last

        # ------------------------------------------------------------------
        for L in layers:
            li, DIN, KCH = L["li"], L["din"], L["kch"]
            H, C, HC, NLIN, NW = L["h"], L["c"], L["hc"], L["nlin"], L["nw"]
            HB, ESF, PWB, SC = L["hb"], L["esf"], L["pwb"], L["sc"]
            hdt = fp8e4 if L["fp8"] else bf
            PWE = PWB // (1 if L["fp8"] else 2)   # payload row in hdt elems

            # layer constants
            wt = [waugp.tile([128, NW], bf, tag=f"waug_kc{k}", name=f"waug_kc{k}") for k in range(KCH)]
            for k in range(KCH):
                kk = min(128, DIN - k * 128)
                nc.sync.dma_start(out=wt[k][:kk, :], in_=waug_p[li][k * 128 : k * 128 + kk, :])
            bsum = bsump.tile([128, NLIN], f32, tag="bsum")
            nc.sync.dma_start(out=bsum[:, :], in_=bsum_p[li][:, :])
            edt_all = edtp.tile([128, T * HMAX], f32, tag="edt")

            # ---------------- dense phase --------------------------------
            for t in range(T):
                r = rows_of(t)
                pd = psum_d.tile([128, NW], f32, tag="pd", name="pd")
                for k in range(KCH):
                    kk = min(128, DIN - k * 128)
                    lhsT = xtp.tile([128, 128], bf, tag="lhsT", name="lhsT")
                    if li == 1:
                        nc.sync.dma_start(out=lhsT[:kk, :], in_=xT1[:, t * 128 : (t + 1) * 128])
                    else:
                        nc.sync.dma_start(
                            out=lhsT[:, :],
                            in_=xrows[li - 1][t * 128 : (t + 1) * 128, k * 128 : (k + 1) * 128],
                            transpose=True,
                        )
                    for nb in range(math.ceil(NW / 512)):
                        w = min(512, NW - nb * 512)
                        nc.tensor.matmul(
                            pd[:, nb * 512 : nb * 512 + w],
                            lhsT[:kk, :],
                            wt[k][:kk, nb * 512 : nb * 512 + w],
                            start=(k == 0),
                            stop=(k == KCH - 1),
                        )
                # stash ed for the aggregation phase (SBUF-resident)
                nc.vector.tensor_copy(edt_all[:, t * HMAX : t * HMAX + H],
                                      pd[:, HC + NLIN + H : HC + NLIN + 2 * H])
                # payload: h (scaled, fp8/bf16) + es f32 tail
                pt = ptp.tile([128, PWB // 2], u16, tag="pt")
                if HB < ESF * 4:
                    nc.vector.memset(pt[:, HB // 2 : ESF * 2], 0.0)
                if ESF * 4 + 4 * H < PWB:
                    nc.vector.memset(pt[:, ESF * 2 + 2 * H :], 0.0)
                hview = pt.bitcast(hdt)[:, 0:HC]
                nc.vector.tensor_scalar(out=hview, in0=pd[:, 0:HC],
                                        scalar1=SC, scalar2=None, op0=ALU.mult)
                ptf = pt.bitcast(f32)
                nc.vector.tensor_copy(ptf[:, ESF : ESF + H], pd[:, HC + NLIN : HC + NLIN + H])
                nc.sync.dma_start(out=pshard[li][t * 128 : (t + 1) * 128, :], in_=pt[:, :])
                # lin + bias
                lt = ltp.tile([128, NLIN], bf, tag="lt")
                nc.vector.tensor_tensor(out=lt[:, :], in0=pd[:, HC : HC + NLIN], in1=bsum[:, :], op=ALU.add)
                nc.sync.dma_start(out=linb[li][t * 128 : (t + 1) * 128, :], in_=lt[:, :])

            # ---------------- all-gather ---------------------------------
            nc.gpsimd.collective_compute(
                "AllGather",
                ALU.bypass,
                replica_groups=[list(range(n_cores))],
                ins=[pshard[li].opt()],
                outs=[pfull[li].opt()],
            )

            # ---------------- aggregation phase --------------------------
            for t in range(T):
                r = rows_of(t)
                K = K_t[t]
                s16 = idxp.tile([128, K_max * 8], i16, tag="s16")
                msk = idxp.tile([128, K_max], f32, tag="msk")
                nc.sync.dma_start(out=s16[:, : K * 8], in_=src16_p[:, K_off[t] * 8 : (K_off[t] + K) * 8])
                nc.sync.dma_start(out=msk[:, :K], in_=mask_p[:, K_off[t] : K_off[t] + K])
                sG = sgp.tile([128, PWB // 2], u16, tag="sG")
                nc.sync.dma_start(out=sG[:, :], in_=pshard[li][t * 128 : (t + 1) * 128, :])
                GRP = 6        # chunks per gather group (768 idxs: HW limit <1024)
                NG = math.ceil(K / GRP)
                Gs = []
                for gi in range(NG):
                    kg = min(GRP, K - gi * GRP)
                    Gt = gp.tile([128, GRP * PWB // 2], u16, tag="G", name=f"G{gi}")
                    nc.gpsimd.dma_gather(
                        out_ap=Gt[:, : kg * PWB // 2].rearrange("p (k w) -> p k w", k=kg),
                        in_ap=pfull[li][:, :],
                        idxs_ap=s16[:, gi * GRP * 8 : (gi * GRP + kg) * 8],
                        num_idxs=kg * 128,
                        num_idxs_reg=kg * 128,
                        elem_size=PWB // 2,
                    )
                    Gs.append((Gt, kg))
                sGf = sG.bitcast(f32)
                sGh = sG.bitcast(hdt)
                edt = edt_all[:, t * HMAX : t * HMAX + H]

                # logits: tl = es[src] + ed[dst] (+mask), lrelu, exp
                wea = wep.tile([128, (K_max + 1) * H], f32, tag="wea")
                tl = wep.tile([128, (K_max + 1) * H], f32, tag="tl")
                for gi, (Gt, kg) in enumerate(Gs):
                    Gf = Gt[:, : kg * PWB // 2].bitcast(f32).rearrange("p (k w) -> p k w", k=kg)
                    tl3 = tl[:, gi * GRP * H : (gi * GRP + kg) * H].rearrange("p (k h) -> p k h", k=kg)
                    nc.vector.tensor_tensor(
                        out=tl3, in0=Gf[:, :, ESF : ESF + H],
                        in1=edt.rearrange("p (k h) -> p k h", k=1).broadcast_to([128, kg, H]),
                        op=ALU.add,
                    )
                nc.vector.tensor_tensor(
                    out=tl[:, : K * H].rearrange("p (k h) -> p k h", k=K),
                    in0=tl[:, : K * H].rearrange("p (k h) -> p k h", k=K),
                    in1=msk[:, :K].rearrange("p (k h) -> p k h", h=1).broadcast_to([128, K, H]),
                    op=ALU.add,
                )
                nc.vector.tensor_tensor(
                    out=tl[:, K * H : (K + 1) * H], in0=sGf[:, ESF : ESF + H],
                    in1=edt, op=ALU.add,
                )
                nc.vector.scalar_tensor_tensor(
                    out=tl[:, : (K + 1) * H], in0=tl[:, : (K + 1) * H],
                    scalar=0.2, in1=tl[:, : (K + 1) * H],
                    op0=ALU.mult, op1=ALU.max,
                )
                nc.scalar.activation(wea[:, : (K + 1) * H], tl[:, : (K + 1) * H], EXP)

                # denominator -> reciprocal (payload scale folded in)
                den = recp.tile([128, HMAX], f32, tag="den")
                nc.vector.reduce_sum(
                    den[:, :H],
                    wea[:, : (K + 1) * H].rearrange("p (k h) -> p h k", h=H),
                    axis=AX,
                )
                rec = recp.tile([128, HMAX], f32, tag="rec")
                nc.vector.tensor_scalar(out=den[:, :H], in0=den[:, :H],
                                        scalar1=SC, scalar2=None, op0=ALU.mult)
                nc.vector.reciprocal(rec[:, :H], den[:, :H])

                # weighted segment-sum via diagonal matmuls (head-outer so
                # each head's PSUM accumulation group closes before the next
                # opens in the same bank)
                CP = math.ceil(C / 128) * 128     # per-head psum col stride
                ps = psum_a.tile([128, H * CP], f32, tag="ps", name="ps")
                wea3 = wea[:, : (K + 1) * H].rearrange("p (k h) -> p k h", h=H)
                n_act = 1 if H == 4 else 2        # heads whose diag builds go to ACT
                for h in range(H):
                    dgh = dgp.tile([128, (K_max + 1) * 128], bf, tag="dgh")
                    if h < n_act:
                        for j in range(K + 1):
                            nc.scalar.activation(
                                dgh[:, j * 128 : (j + 1) * 128], ident[:, :], COPY,
                                scale=wea[:, j * H + h : j * H + h + 1],
                            )
                    else:
                        nc.vector.tensor_tensor(
                            out=dgh[:, : (K + 1) * 128].rearrange("p (k q) -> p k q", k=K + 1),
                            in0=ident.rearrange("p (k q) -> p k q", k=1).broadcast_to([128, K + 1, 128]),
                            in1=wea3[:, :, h : h + 1].broadcast_to([128, K + 1, 128]),
                            op=ALU.mult,
                        )
                    for j in range(K + 1):        # j==K is the self chunk
                        if j == K:
                            rhs = sGh[:, h * C : (h + 1) * C]
                        else:
                            Gt, _ = Gs[j // GRP]
                            jj = j % GRP
                            rhs = Gt.bitcast(hdt)[:, jj * PWE + h * C : jj * PWE + (h + 1) * C]
                        nc.tensor.matmul(
                            ps[:, h * CP : h * CP + C],
                            dgh[:, j * 128 : (j + 1) * 128],
                            rhs,
                            start=(j == 0),
                            stop=(j == K),
                        )

                # epilogue
                xt = epip.tile([128, HC], f32, tag="xt")
                for h in range(H):
                    if h % 2 == 0:
                        nc.scalar.activation(
                            xt[:, h * C : (h + 1) * C], ps[:, h * CP : h * CP + C],
                            COPY, scale=rec[:, h : h + 1],
                        )
                    else:
                        nc.vector.tensor_scalar(
                            out=xt[:, h * C : (h + 1) * C], in0=ps[:, h * CP : h * CP + C],
                            scalar1=rec[:, h : h + 1], scalar2=None, op0=ALU.mult,
                        )
                lt2 = ltp.tile([128, NLIN], bf, tag="lt2")
                nc.sync.dma_start(out=lt2[:, :], in_=linb[li][t * 128 : (t + 1) * 128, :])
                if li < 3:
                    s = epip.tile([128, HC], f32, tag="s")
                    u = epip.tile([128, HC], f32, tag="u")
                    e = epip.tile([128, HC], f32, tag="e")
                    xo = epip.tile([128, HC], bf, tag="xo")
                    nc.vector.tensor_tensor(out=s[:, :], in0=xt[:, :], in1=lt2[:, :], op=ALU.add)
                    nc.vector.tensor_scalar(out=u[:, :], in0=s[:, :], scalar1=0.0, scalar2=None, op0=ALU.min)
                    nc.scalar.activation(e[:, :], u[:, :], EXP)
                    # elu(s) = relu(s) + exp(min(s,0)) - 1
                    nc.vector.scalar_tensor_tensor(
                        out=s[:, :], in0=s[:, :], scalar=0.0, in1=e[:, :],
                        op0=ALU.max, op1=ALU.add,
                    )
                    nc.vector.tensor_scalar(out=xo[:, :], in0=s[:, :],
                                            scalar1=-1.0, scalar2=None, op0=ALU.add)
                    nc.sync.dma_start(out=xrows[li][t * 128 : (t + 1) * 128, :], in_=xo[:, :])
                else:
                    xt3 = xt.rearrange("p (h c) -> p h c", h=H)
                    m1 = epip.tile([128, 3, C], f32, tag="m1")
                    nc.vector.tensor_tensor(out=m1[:, :, :], in0=xt3[:, 0:3, :], in1=xt3[:, 3:6, :], op=ALU.add)
                    m2 = epip.tile([128, C], f32, tag="m2")
                    nc.vector.tensor_tensor(out=m2[:, :], in0=m1[:, 0, :], in1=m1[:, 1, :], op=ALU.add)
                    nc.vector.tensor_tensor(out=m2[:, :], in0=m2[:, :], in1=m1[:, 2, :], op=ALU.add)
                    ot = epip.tile([128, C], f32, tag="ot")
                    nc.vector.scalar_tensor_tensor(
                        out=ot[:r, :], in0=m2[:r, :], scalar=1.0 / H, in1=lt2[:r, :],
                        op0=ALU.mult, op1=ALU.add,
                    )
                    nc.sync.dma_start(out=out_p[t * 128 : t * 128 + r, :], in_=ot[:r, :])

    nc.finalize()
    return nc


# --------------------------------------------------------------------------
# runner
# --------------------------------------------------------------------------

def _run(inputs, sim=False, trace=False, n_cores=N_CORES, tmpdir=None):
    in_maps, cfg, perm = _host_prep(inputs, n_cores)
    nc = _build(cfg)
    if sim:
        import concourse.bass_interp as bass_interp

        msim = bass_interp.MultiCoreSim(nc, n_cores)
        for c in range(n_cores):
            for k, v in in_maps[c].items():
                msim.cores[c].tensor(k)[:] = v
        msim.simulate(check_with_hw=False)
        outs = [np.array(msim.cores[c].mem_tensor("out")) for c in range(n_cores)]
        exec_ns = None
    else:
        from concourse.bass_utils import run_bass_kernel_spmd

        res = run_bass_kernel_spmd(
            nc, in_maps, list(range(n_cores)), trace=trace, tmpdir=tmpdir
        )
        outs = [res.results[c]["out"] for c in range(n_cores)]
        exec_ns = res.exec_time_ns
    out_new = np.concatenate(outs, 0)
    out = np.empty_like(out_new)
    out[...] = out_new[perm]
    return out.astype(np.float32), exec_ns


def kernel(**inputs) -> np.ndarray:
    out, _ = _run(inputs)
    return out


# revision 59
# speedup vs baseline: 1.0747x; 1.0747x over previous
"""3-layer GAT (PPI-style) forward on 8 Trainium2 NeuronCores.

Strategy (SPMD, one NEFF on 8 cores):
  - Host: degree-banded node permutation: nodes sorted by in-degree, dealt
    round-robin into 8 cores x 20 tiles of 128 dst rows, so every tile-slot t
    has the same per-row slot count K_t on all cores (<5% slot padding).
  - dst-ALIGNED edge layout: partition p of tile t owns dst row p; its
    incoming edges occupy slots j=0..deg-1 (chunk j).  Aggregation is then
    psum[p, :] += we[p,j] * G[p, j, :] done as PE matmuls with DIAGONAL
    lhsT = diag(we[:, j, h]) -- no one-hot builds, no per-edge dst gather.
  - Self-loops excluded from the gather; handled via one contiguous DMA of
    the tile's own payload rows (pshard) as an extra chunk.
  - Attention dots es/ed folded into the dense matmul on the host:
    waug = [W | Wl | W@a_s | W@a_d]; es/ed come out as 2H extra psum cols.
  - Payload per node: h in fp8e4 (scaled 1/8) + es in f32 tail; AllGather
    payload across cores; per-edge gather of 1280B (L1/2) / 768B (L3) rows.
  - exp(leakyrelu(es+ed)) exact softmax (no max-subtraction; |t| < ~9),
    padding slots killed via -1e30 mask added to the logit.
"""

import math
import numpy as np

N_CORES = 8
FP8L = {1: False, 2: False, 3: False}  # payload h dtype per layer: fp8e4 vs bf16
STRIDE0 = True      # batched stride-0 broadcast DVE ops (diag build etc.)


# --------------------------------------------------------------------------
# host-side prep (pure data layout / graph partitioning, no model math)
# --------------------------------------------------------------------------

def _wrap16_rep(a):
    """[L] int -> [128, L/16] int16 (16-wrap, replicated 8x down partitions)."""
    w = a.reshape(-1, 16).T.astype(np.int16)
    return np.ascontiguousarray(np.tile(w, (8, 1)))


def _host_prep(inputs, n_cores=N_CORES):
    import ml_dtypes

    bf16 = ml_dtypes.bfloat16
    x = np.asarray(inputs["x"], np.float32)
    ei = np.asarray(inputs["edge_index"])
    n, f_in = x.shape
    src = ei[0].astype(np.int64)
    dst = ei[1].astype(np.int64)

    per_core = n // n_cores                      # 2500
    T = math.ceil(per_core / 128)                # 20
    rows_last = per_core - (T - 1) * 128         # 68

    # ---- degree-banded permutation (self-loops handled separately) -------
    rows_pad = T * 128
    deg = np.bincount(dst, minlength=n).astype(np.int64)
    order = np.argsort(-deg, kind="stable")
    perm = np.empty(n, np.int64)       # output-row space (per_core rows/core)
    perm_pay = np.empty(n, np.int64)   # payload-row space (rows_pad rows/core)
    K_t = []
    pos = 0
    for t in range(T):
        rows = 128 if t < T - 1 else rows_last
        band = order[pos : pos + rows * n_cores]
        pos += rows * n_cores
        K_t.append(max(1, int(deg[band].max())))
        idx = np.arange(band.shape[0])
        c = idx % n_cores
        r = idx // n_cores
        perm[band] = c * per_core + t * 128 + r
        perm_pay[band] = c * rows_pad + t * 128 + r
    K_off = np.concatenate([[0], np.cumsum(K_t)]).astype(np.int64)
    K_sum = int(K_off[-1])

    src_n = perm_pay[src]              # gather indices -> payload rows
    dst_n = perm[dst]

    # ---- per-core slot arrays -------------------------------------------
    src16_list, mask_list = [], []
    core_of = dst_n // per_core
    for c in range(n_cores):
        sel = core_of == c
        s, d = src_n[sel], dst_n[sel]
        loc = d - c * per_core
        t_of = loc // 128
        r_of = loc - t_of * 128
        flat_parts = []
        mask = np.full((128, K_sum), -300.0, np.float32)
        for t in range(T):
            K = K_t[t]
            m = t_of == t
            rr = r_of[m]
            ss = s[m]
            o = np.argsort(rr, kind="stable")
            rr, ss = rr[o], ss[o]
            flat = np.zeros(K * 128, np.int64)
            # occurrence index per row
            occ = np.zeros_like(rr)
            if rr.size:
                chg = np.concatenate([[True], rr[1:] != rr[:-1]])
                idx0 = np.flatnonzero(chg)
                occ = np.arange(rr.size) - np.repeat(idx0, np.diff(np.concatenate([idx0, [rr.size]])))
            flat[occ * 128 + rr] = ss
            mask[rr, K_off[t] + occ] = 0.0
            flat_parts.append(flat)
        src16_list.append(np.concatenate([_wrap16_rep(f) for f in flat_parts], axis=1))
        mask_list.append(np.ascontiguousarray(mask))

    # ---- permuted node features, transposed, padded rows, bf16, per core
    x_perm = np.zeros((n, f_in), np.float32)
    x_perm[perm] = x
    xT = []
    for c in range(n_cores):
        blk = np.zeros((rows_pad, f_in), np.float32)
        blk[:per_core] = x_perm[c * per_core : (c + 1) * per_core]
        xT.append(np.ascontiguousarray(blk.T).astype(bf16))

    # ---- weights: waug = [W | Wl | W@a_s | W@a_d], bias pre-summed -------
    g = lambda k: np.asarray(inputs[k], np.float32)
    rep = lambda v: np.ascontiguousarray(np.broadcast_to(v[None, :], (128, v.shape[0]))).astype(np.float32)

    def fold(Wk, Wlk, ask, adk):
        W, Wl = g(Wk), g(Wlk)
        a_s, a_d = g(ask), g(adk)
        h_, c_ = a_s.shape
        din = W.shape[0]
        vs = np.stack([W[:, i * c_ : (i + 1) * c_] @ a_s[i] for i in range(h_)], 1)
        vd = np.stack([W[:, i * c_ : (i + 1) * c_] @ a_d[i] for i in range(h_)], 1)
        return np.ascontiguousarray(np.concatenate([W, Wl, vs, vd], 1)).astype(bf16)

    waug1 = fold("W1", "Wl1", "a1s", "a1d")      # [50, 2056]
    waug2 = fold("W2", "Wl2", "a2s", "a2d")      # [1024, 2056]
    waug3 = fold("W3", "Wl3", "a3s", "a3d")      # [1024, 859]

    base = dict(
        waug1=waug1, waug2=waug2, waug3=waug3,
        bsum1=rep(g("b1") + g("bl1")),
        bsum2=rep(g("b2") + g("bl2")),
        bsum3=rep(g("b3") + g("bl3")),
    )
    in_maps = []
    for c in range(n_cores):
        m = dict(base)
        m["xT1"] = xT[c]
        m["src16"] = src16_list[c]
        m["maskneg"] = mask_list[c]
        in_maps.append(m)

    h1, c1 = np.asarray(inputs["a1s"]).shape
    h3, c3 = np.asarray(inputs["a3s"]).shape
    cfg = dict(
        n=n, f_in=f_in, n_cores=n_cores, per_core=per_core,
        T=T, rows_last=rows_last, rows_pad=rows_pad,
        K_t=K_t, K_off=[int(v) for v in K_off], K_sum=K_sum,
        h1=h1, c1=c1, d1=h1 * c1, h3=h3, c3=c3,
    )
    return in_maps, cfg, perm


# --------------------------------------------------------------------------
# bass program
# --------------------------------------------------------------------------

def _layer_dims(cfg):
    out = []
    for li in (1, 2, 3):
        if li < 3:
            h, c = cfg["h1"], cfg["c1"]
            din = cfg["f_in"] if li == 1 else cfg["d1"]
            nlin = cfg["d1"]
        else:
            h, c = cfg["h3"], cfg["c3"]
            din = cfg["d1"]
            nlin = cfg["c3"]
        hc = h * c
        nw = hc + nlin + 2 * h                 # psum cols: h | lin | es | ed
        fp8 = FP8L[li]
        psz = 1 if fp8 else 2                  # payload h dtype size
        hb = hc * psz                          # h bytes in payload
        esb = math.ceil(hb / 8) * 8            # es byte offset (8-align)
        pwb = math.ceil((esb + 4 * h) / 256) * 256   # payload bytes
        kch = math.ceil(din / 128)
        out.append(dict(li=li, din=din, kch=kch, h=h, c=c, hc=hc, nlin=nlin,
                        nw=nw, hb=hb, esf=esb // 4, pwb=pwb, fp8=fp8,
                        sc=0.125 if fp8 else 1.0))
    return out


def _build(cfg):
    import concourse.bass as bass
    import concourse.bacc as bacc
    import concourse.mybir as mybir
    import concourse.tile as tile
    from contextlib import ExitStack

    f32 = mybir.dt.float32
    bf = mybir.dt.bfloat16
    i16 = mybir.dt.int16
    i32 = mybir.dt.int32
    u8 = mybir.dt.uint8
    fp8e4 = mybir.dt.float8e4
    EXP = mybir.ActivationFunctionType.Exp
    COPY = mybir.ActivationFunctionType.Copy
    ALU = mybir.AluOpType
    AX = mybir.AxisListType.X

    n_cores = cfg["n_cores"]
    n = cfg["n"]
    T = cfg["T"]
    rows_last = cfg["rows_last"]
    per_core = cfg["per_core"]
    rows_pad = cfg["rows_pad"]
    K_t = cfg["K_t"]
    K_off = cfg["K_off"]
    K_sum = cfg["K_sum"]
    K_max = max(K_t)
    D1 = cfg["d1"]
    layers = _layer_dims(cfg)
    HMAX = max(L["h"] for L in layers)

    nc = bacc.Bacc(None, target_bir_lowering=False)

    # ---- parameters -----------------------------------------------------
    xT1 = nc.declare_dram_parameter("xT1", [cfg["f_in"], rows_pad], bf, isOutput=False)
    waug_p = {L["li"]: nc.declare_dram_parameter(f"waug{L['li']}", [L["din"], L["nw"]], bf, isOutput=False)
              for L in layers}
    bsum_p = {L["li"]: nc.declare_dram_parameter(f"bsum{L['li']}", [128, L["nlin"]], f32, isOutput=False)
              for L in layers}
    src16_p = nc.declare_dram_parameter("src16", [128, K_sum * 8], i16, isOutput=False)
    mask_p = nc.declare_dram_parameter("maskneg", [128, K_sum], f32, isOutput=False)
    out_p = nc.declare_dram_parameter("out", [per_core, cfg["c3"]], f32, isOutput=True)

    with tile.TileContext(nc, num_cores=n_cores) as tc, ExitStack() as ctx:
        # ---- dram scratch ----------------------------------------------
        dram = ctx.enter_context(tc.tile_pool(name="dram", bufs=1, space="DRAM"))
        u16 = mybir.dt.uint16
        pshard = {L["li"]: dram.tile([rows_pad, L["pwb"] // 2], u16, tag=f"pshard{L['li']}", name=f"pshard{L['li']}")
                  for L in layers}
        pfull = {L["li"]: dram.tile([n_cores * rows_pad, L["pwb"] // 2], u16, tag=f"pfull{L['li']}", name=f"pfull{L['li']}",
                                    addr_space="Shared") for L in layers}
        xrows = {li: dram.tile([rows_pad, D1], bf, tag=f"xrows{li}", name=f"xrows{li}") for li in (1, 2)}
        linb = {L["li"]: dram.tile([rows_pad, L["nlin"]], bf, tag=f"lin{L['li']}", name=f"lin{L['li']}")
                for L in layers}

        # ---- pools ------------------------------------------------------
        consts = ctx.enter_context(tc.tile_pool(name="consts", bufs=1))
        waugp = ctx.enter_context(tc.tile_pool(name="waugp", bufs=1))
        bsump = ctx.enter_context(tc.tile_pool(name="bsump", bufs=1))
        edtp = ctx.enter_context(tc.tile_pool(name="edtp", bufs=1))
        xtp = ctx.enter_context(tc.tile_pool(name="xtp", bufs=3))
        ptp = ctx.enter_context(tc.tile_pool(name="ptp", bufs=3))
        ltp = ctx.enter_context(tc.tile_pool(name="ltp", bufs=2))
        gp = ctx.enter_context(tc.tile_pool(name="gp", bufs=8))
        sgp = ctx.enter_context(tc.tile_pool(name="sgp", bufs=2))
        idxp = ctx.enter_context(tc.tile_pool(name="idxp", bufs=2))
        wep = ctx.enter_context(tc.tile_pool(name="wep", bufs=2))
        dgp = ctx.enter_context(tc.tile_pool(name="dgp", bufs=2))
        epip = ctx.enter_context(tc.tile_pool(name="epip", bufs=1))
        recp = ctx.enter_context(tc.tile_pool(name="recp", bufs=4))
        psum_d = ctx.enter_context(tc.tile_pool(name="psum_d", bufs=1, space="PSUM"))
        psum_a = ctx.enter_context(tc.tile_pool(name="psum_a", bufs=1, space="PSUM"))

        # ---- constants ---------------------------------------------------
        iota_i = consts.tile([128, 128], i32, tag="iota_i")
        nc.gpsimd.iota(iota_i[:, :], pattern=[[1, 128]], base=0, channel_multiplier=0)
        pidx_i = consts.tile([128, 1], i32, tag="pidx_i")
        nc.gpsimd.iota(pidx_i[:, :], pattern=[[1, 1]], base=0, channel_multiplier=1)
        iota_f = consts.tile([128, 128], f32, tag="iota_f")
        nc.vector.tensor_copy(iota_f[:, :], iota_i[:, :])
        pidx_f = consts.tile([128, 1], f32, tag="pidx_f")
        nc.vector.tensor_copy(pidx_f[:, :], pidx_i[:, :])
        ident = consts.tile([128, 128], bf, tag="ident")
        nc.vector.tensor_scalar(out=ident[:, :], in0=iota_f[:, :],
                                scalar1=pidx_f[:, 0:1], scalar2=None, op0=ALU.is_equal)

        def rows_of(t):
            return 128 if t < T - 1 else rows_last

        # ------------------------------------------------------------------
        for L in layers:
            li, DIN, KCH = L["li"], L["din"], L["kch"]
            H, C, HC, NLIN, NW = L["h"], L["c"], L["hc"], L["nlin"], L["nw"]
            HB, ESF, PWB, SC = L["hb"], L["esf"], L["pwb"], L["sc"]
            hdt = fp8e4 if L["fp8"] else bf
            PWE = PWB // (1 if L["fp8"] else 2)   # payload row in hdt elems

            # layer constants
            wt = [waugp.tile([128, NW], bf, tag=f"waug_kc{k}", name=f"waug_kc{k}") for k in range(KCH)]
            for k in range(KCH):
                kk = min(128, DIN - k * 128)
                nc.sync.dma_start(out=wt[k][:kk, :], in_=waug_p[li][k * 128 : k * 128 + kk, :])
            bsum = bsump.tile([128, NLIN], f32, tag="bsum")
            nc.sync.dma_start(out=bsum[:, :], in_=bsum_p[li][:, :])
            edt_all = edtp.tile([128, T * HMAX], f32, tag="edt")

            # ---------------- dense phase --------------------------------
            for t in range(T):
                r = rows_of(t)
                pd = psum_d.tile([128, NW], f32, tag="pd", name="pd")
                for k in range(KCH):
                    kk = min(128, DIN - k * 128)
                    lhsT = xtp.tile([128, 128], bf, tag="lhsT", name="lhsT")
                    if li == 1:
                        nc.sync.dma_start(out=lhsT[:kk, :], in_=xT1[:, t * 128 : (t + 1) * 128])
                    else:
                        nc.sync.dma_start(
                            out=lhsT[:, :],
                            in_=xrows[li - 1][t * 128 : (t + 1) * 128, k * 128 : (k + 1) * 128],
                            transpose=True,
                        )
                    for nb in range(math.ceil(NW / 512)):
                        w = min(512, NW - nb * 512)
                        nc.tensor.matmul(
                            pd[:, nb * 512 : nb * 512 + w],
                            lhsT[:kk, :],
                            wt[k][:kk, nb * 512 : nb * 512 + w],
                            start=(k == 0),
                            stop=(k == KCH - 1),
                        )
                # stash ed for the aggregation phase (SBUF-resident)
                nc.vector.tensor_copy(edt_all[:, t * HMAX : t * HMAX + H],
                                      pd[:, HC + NLIN + H : HC + NLIN + 2 * H])
                # payload: h (scaled, fp8/bf16) + es f32 tail
                pt = ptp.tile([128, PWB // 2], u16, tag="pt")
                if HB < ESF * 4:
                    nc.vector.memset(pt[:, HB // 2 : ESF * 2], 0.0)
                if ESF * 4 + 4 * H < PWB:
                    nc.vector.memset(pt[:, ESF * 2 + 2 * H :], 0.0)
                hview = pt.bitcast(hdt)[:, 0:HC]
                nc.vector.tensor_scalar(out=hview, in0=pd[:, 0:HC],
                                        scalar1=SC, scalar2=None, op0=ALU.mult)
                ptf = pt.bitcast(f32)
                nc.vector.tensor_copy(ptf[:, ESF : ESF + H], pd[:, HC + NLIN : HC + NLIN + H])
                nc.sync.dma_start(out=pshard[li][t * 128 : (t + 1) * 128, :], in_=pt[:, :])
                # lin + bias
                lt = ltp.tile([128, NLIN], bf, tag="lt")
                nc.vector.tensor_tensor(out=lt[:, :], in0=pd[:, HC : HC + NLIN], in1=bsum[:, :], op=ALU.add)
                nc.sync.dma_start(out=linb[li][t * 128 : (t + 1) * 128, :], in_=lt[:, :])

            # ---------------- all-gather ---------------------------------
            nc.gpsimd.collective_compute(
                "AllGather",
                ALU.bypass,
                replica_groups=[list(range(n_cores))],
                ins=[pshard[li].opt()],
                outs=[pfull[li].opt()],
            )

            # ---------------- aggregation phase --------------------------
            for t in range(T):
                r = rows_of(t)
                K = K_t[t]
                s16 = idxp.tile([128, K_max * 8], i16, tag="s16")
                msk = idxp.tile([128, K_max], f32, tag="msk")
                nc.sync.dma_start(out=s16[:, : K * 8], in_=src16_p[:, K_off[t] * 8 : (K_off[t] + K) * 8])
                nc.sync.dma_start(out=msk[:, :K], in_=mask_p[:, K_off[t] : K_off[t] + K])
                sG = sgp.tile([128, PWB // 2], u16, tag="sG")
                nc.sync.dma_start(out=sG[:, :], in_=pshard[li][t * 128 : (t + 1) * 128, :])
                GRP = 6        # chunks per gather group (768 idxs: HW limit <1024)
                NG = math.ceil(K / GRP)
                Gs = []
                for gi in range(NG):
                    kg = min(GRP, K - gi * GRP)
                    Gt = gp.tile([128, GRP * PWB // 2], u16, tag="G", name=f"G{gi}")
                    nc.gpsimd.dma_gather(
                        out_ap=Gt[:, : kg * PWB // 2].rearrange("p (k w) -> p k w", k=kg),
                        in_ap=pfull[li][:, :],
                        idxs_ap=s16[:, gi * GRP * 8 : (gi * GRP + kg) * 8],
                        num_idxs=kg * 128,
                        num_idxs_reg=kg * 128,
                        elem_size=PWB // 2,
                    )
                    Gs.append((Gt, kg))
                sGf = sG.bitcast(f32)
                sGh = sG.bitcast(hdt)
                edt = edt_all[:, t * HMAX : t * HMAX + H]

                # logits: tl = es[src] + ed[dst] (+mask), lrelu, exp
                wea = wep.tile([128, (K_max + 1) * H], f32, tag="wea")
                tl = wep.tile([128, (K_max + 1) * H], f32, tag="tl")
                for gi, (Gt, kg) in enumerate(Gs):
                    Gf = Gt[:, : kg * PWB // 2].bitcast(f32).rearrange("p (k w) -> p k w", k=kg)
                    tl3 = tl[:, gi * GRP * H : (gi * GRP + kg) * H].rearrange("p (k h) -> p k h", k=kg)
                    nc.vector.tensor_tensor(
                        out=tl3, in0=Gf[:, :, ESF : ESF + H],
                        in1=edt.rearrange("p (k h) -> p k h", k=1).broadcast_to([128, kg, H]),
                        op=ALU.add,
                    )
                nc.vector.tensor_tensor(
                    out=tl[:, : K * H].rearrange("p (k h) -> p k h", k=K),
                    in0=tl[:, : K * H].rearrange("p (k h) -> p k h", k=K),
                    in1=msk[:, :K].rearrange("p (k h) -> p k h", h=1).broadcast_to([128, K, H]),
                    op=ALU.add,
                )
                nc.vector.tensor_tensor(
                    out=tl[:, K * H : (K + 1) * H], in0=sGf[:, ESF : ESF + H],
                    in1=edt, op=ALU.add,
                )
                nc.vector.scalar_tensor_tensor(
                    out=tl[:, : (K + 1) * H], in0=tl[:, : (K + 1) * H],
                    scalar=0.2, in1=tl[:, : (K + 1) * H],
                    op0=ALU.mult, op1=ALU.max,
                )
                nc.scalar.activation(wea[:, : (K + 1) * H], tl[:, : (K + 1) * H], EXP)

                # denominator -> reciprocal (payload scale folded in)
                den = recp.tile([128, HMAX], f32, tag="den")
                nc.vector.reduce_sum(
                    den[:, :H],
                    wea[:, : (K + 1) * H].rearrange("p (k h) -> p h k", h=H),
                    axis=AX,
                )
                rec = recp.tile([128, HMAX], f32, tag="rec")
                nc.vector.tensor_scalar(out=den[:, :H], in0=den[:, :H],
                                        scalar1=SC, scalar2=None, op0=ALU.mult)
                nc.vector.reciprocal(rec[:, :H], den[:, :H])

                # weighted segment-sum via diagonal matmuls (head-outer so
                # each head's PSUM accumulation group closes before the next
                # opens in the same bank)
                CP = math.ceil(C / 128) * 128     # per-head psum col stride
                ps = psum_a.tile([128, H * CP], f32, tag="ps", name="ps")
                wea3 = wea[:, : (K + 1) * H].rearrange("p (k h) -> p k h", h=H)
                n_act = 0        # heads whose diag builds go to ACT
                for h in range(H):
                    dgh = dgp.tile([128, (K_max + 1) * 128], bf, tag="dgh")
                    if h < n_act:
                        for j in range(K + 1):
                            nc.scalar.activation(
                                dgh[:, j * 128 : (j + 1) * 128], ident[:, :], COPY,
                                scale=wea[:, j * H + h : j * H + h + 1],
                            )
                    else:
                        nc.vector.tensor_tensor(
                            out=dgh[:, : (K + 1) * 128].rearrange("p (k q) -> p k q", k=K + 1),
                            in0=ident.rearrange("p (k q) -> p k q", k=1).broadcast_to([128, K + 1, 128]),
                            in1=wea3[:, :, h : h + 1].broadcast_to([128, K + 1, 128]),
                            op=ALU.mult,
                        )
                    for j in range(K + 1):        # j==K is the self chunk
                        if j == K:
                            rhs = sGh[:, h * C : (h + 1) * C]
                        else:
                            Gt, _ = Gs[j // GRP]
                            jj = j % GRP
                            rhs = Gt.bitcast(hdt)[:, jj * PWE + h * C : jj * PWE + (h + 1) * C]
                        nc.tensor.matmul(
                            ps[:, h * CP : h * CP + C],
                            dgh[:, j * 128 : (j + 1) * 128],
                            rhs,
                            start=(j == 0),
                            stop=(j == K),
                        )

                # epilogue
                xt = epip.tile([128, HC], f32, tag="xt")
                for h in range(H):
                    if False:
                        nc.scalar.activation(
                            xt[:, h * C : (h + 1) * C], ps[:, h * CP : h * CP + C],
                            COPY, scale=rec[:, h : h + 1],
                        )
                    else:
                        nc.vector.tensor_scalar(
                            out=xt[:, h * C : (h + 1) * C], in0=ps[:, h * CP : h * CP + C],
                            scalar1=rec[:, h : h + 1], scalar2=None, op0=ALU.mult,
                        )
                lt2 = ltp.tile([128, NLIN], bf, tag="lt2")
                nc.sync.dma_start(out=lt2[:, :], in_=linb[li][t * 128 : (t + 1) * 128, :])
                if li < 3:
                    s = epip.tile([128, HC], f32, tag="s")
                    u = epip.tile([128, HC], f32, tag="u")
                    e = epip.tile([128, HC], f32, tag="e")
                    xo = epip.tile([128, HC], bf, tag="xo")
                    nc.vector.tensor_tensor(out=s[:, :], in0=xt[:, :], in1=lt2[:, :], op=ALU.add)
                    nc.vector.tensor_scalar(out=u[:, :], in0=s[:, :], scalar1=0.0, scalar2=None, op0=ALU.min)
                    nc.scalar.activation(e[:, :], u[:, :], EXP)
                    # elu(s) = relu(s) + exp(min(s,0)) - 1
                    nc.vector.scalar_tensor_tensor(
                        out=s[:, :], in0=s[:, :], scalar=0.0, in1=e[:, :],
                        op0=ALU.max, op1=ALU.add,
                    )
                    nc.vector.tensor_scalar(out=xo[:, :], in0=s[:, :],
                                            scalar1=-1.0, scalar2=None, op0=ALU.add)
                    nc.sync.dma_start(out=xrows[li][t * 128 : (t + 1) * 128, :], in_=xo[:, :])
                else:
                    xt3 = xt.rearrange("p (h c) -> p h c", h=H)
                    m1 = epip.tile([128, 3, C], f32, tag="m1")
                    nc.vector.tensor_tensor(out=m1[:, :, :], in0=xt3[:, 0:3, :], in1=xt3[:, 3:6, :], op=ALU.add)
                    m2 = epip.tile([128, C], f32, tag="m2")
                    nc.vector.tensor_tensor(out=m2[:, :], in0=m1[:, 0, :], in1=m1[:, 1, :], op=ALU.add)
                    nc.vector.tensor_tensor(out=m2[:, :], in0=m2[:, :], in1=m1[:, 2, :], op=ALU.add)
                    ot = epip.tile([128, C], f32, tag="ot")
                    nc.vector.scalar_tensor_tensor(
                        out=ot[:r, :], in0=m2[:r, :], scalar=1.0 / H, in1=lt2[:r, :],
                        op0=ALU.mult, op1=ALU.add,
                    )
                    nc.sync.dma_start(out=out_p[t * 128 : t * 128 + r, :], in_=ot[:r, :])

    nc.finalize()
    return nc


# --------------------------------------------------------------------------
# runner
# --------------------------------------------------------------------------

def _run(inputs, sim=False, trace=False, n_cores=N_CORES, tmpdir=None):
    in_maps, cfg, perm = _host_prep(inputs, n_cores)
    nc = _build(cfg)
    if sim:
        import concourse.bass_interp as bass_interp

        msim = bass_interp.MultiCoreSim(nc, n_cores)
        for c in range(n_cores):
            for k, v in in_maps[c].items():
                msim.cores[c].tensor(k)[:] = v
        msim.simulate(check_with_hw=False)
        outs = [np.array(msim.cores[c].mem_tensor("out")) for c in range(n_cores)]
        exec_ns = None
    else:
        from concourse.bass_utils import run_bass_kernel_spmd

        res = run_bass_kernel_spmd(
            nc, in_maps, list(range(n_cores)), trace=trace, tmpdir=tmpdir
        )
        outs = [res.results[c]["out"] for c in range(n_cores)]
        exec_ns = res.exec_time_ns
    out_new = np.concatenate(outs, 0)
    out = np.empty_like(out_new)
    out[...] = out_new[perm]
    return out.astype(np.float32), exec_ns


def kernel(**inputs) -> np.ndarray:
    out, _ = _run(inputs)
    return out


# revision 62
# speedup vs baseline: 1.1559x; 1.0755x over previous
"""3-layer GAT (PPI-style) forward on 8 Trainium2 NeuronCores.

Strategy (SPMD, one NEFF on 8 cores):
  - Host: degree-banded node permutation: nodes sorted by in-degree, dealt
    round-robin into 8 cores x 20 tiles of 128 dst rows, so every tile-slot t
    has the same per-row slot count K_t on all cores (<5% slot padding).
  - dst-ALIGNED edge layout: partition p of tile t owns dst row p; its
    incoming edges occupy slots j=0..deg-1 (chunk j).  Aggregation is then
    psum[p, :] += we[p,j] * G[p, j, :] done as PE matmuls with DIAGONAL
    lhsT = diag(we[:, j, h]) -- no one-hot builds, no per-edge dst gather.
  - Self-loops excluded from the gather; handled via one contiguous DMA of
    the tile's own payload rows (pshard) as an extra chunk.
  - Attention dots es/ed folded into the dense matmul on the host:
    waug = [W | Wl | W@a_s | W@a_d]; es/ed come out as 2H extra psum cols.
  - Payload per node: h in bf16 + es in f32 tail; AllGather payload across
    cores; per-edge gather of 2304B (L1/2) / 1536B (L3) rows, <=768 idxs
    per dma_gather call (HW limit is between 768 and 1024).
  - exp(leakyrelu(es+ed)) exact softmax (no max-subtraction; |t| < ~9),
    padding slots killed via a -300 logit mask (must stay inside the ACT
    exp table's input range; -1e30 returns garbage on HW).
"""

import math
import numpy as np

N_CORES = 8
FP8L = {1: False, 2: False, 3: False}  # payload h dtype per layer: fp8e4 vs bf16
STRIDE0 = True      # batched stride-0 broadcast DVE ops (diag build etc.)


# --------------------------------------------------------------------------
# host-side prep (pure data layout / graph partitioning, no model math)
# --------------------------------------------------------------------------

def _wrap16_rep(a):
    """[L] int -> [128, L/16] int16 (16-wrap, replicated 8x down partitions)."""
    w = a.reshape(-1, 16).T.astype(np.int16)
    return np.ascontiguousarray(np.tile(w, (8, 1)))


def _host_prep(inputs, n_cores=N_CORES):
    import ml_dtypes

    bf16 = ml_dtypes.bfloat16
    x = np.asarray(inputs["x"], np.float32)
    ei = np.asarray(inputs["edge_index"])
    n, f_in = x.shape
    src = ei[0].astype(np.int64)
    dst = ei[1].astype(np.int64)

    per_core = n // n_cores                      # 2500
    T = math.ceil(per_core / 128)                # 20
    rows_last = per_core - (T - 1) * 128         # 68

    # ---- degree-banded permutation (self-loops handled separately) -------
    rows_pad = T * 128
    deg = np.bincount(dst, minlength=n).astype(np.int64)
    order = np.argsort(-deg, kind="stable")
    perm = np.empty(n, np.int64)       # output-row space (per_core rows/core)
    perm_pay = np.empty(n, np.int64)   # payload-row space (rows_pad rows/core)
    K_t = []
    pos = 0
    for t in range(T):
        rows = 128 if t < T - 1 else rows_last
        band = order[pos : pos + rows * n_cores]
        pos += rows * n_cores
        K_t.append(max(1, int(deg[band].max())))
        idx = np.arange(band.shape[0])
        c = idx % n_cores
        r = idx // n_cores
        perm[band] = c * per_core + t * 128 + r
        perm_pay[band] = c * rows_pad + t * 128 + r
    K_off = np.concatenate([[0], np.cumsum(K_t)]).astype(np.int64)
    K_sum = int(K_off[-1])

    src_n = perm_pay[src]              # gather indices -> payload rows
    dst_n = perm[dst]

    # ---- per-core slot arrays -------------------------------------------
    src16_list, mask_list = [], []
    core_of = dst_n // per_core
    for c in range(n_cores):
        sel = core_of == c
        s, d = src_n[sel], dst_n[sel]
        loc = d - c * per_core
        t_of = loc // 128
        r_of = loc - t_of * 128
        flat_parts = []
        mask = np.full((128, K_sum), -300.0, np.float32)
        for t in range(T):
            K = K_t[t]
            m = t_of == t
            rr = r_of[m]
            ss = s[m]
            o = np.argsort(rr, kind="stable")
            rr, ss = rr[o], ss[o]
            flat = np.zeros(K * 128, np.int64)
            # occurrence index per row
            occ = np.zeros_like(rr)
            if rr.size:
                chg = np.concatenate([[True], rr[1:] != rr[:-1]])
                idx0 = np.flatnonzero(chg)
                occ = np.arange(rr.size) - np.repeat(idx0, np.diff(np.concatenate([idx0, [rr.size]])))
            flat[occ * 128 + rr] = ss
            mask[rr, K_off[t] + occ] = 0.0
            flat_parts.append(flat)
        src16_list.append(np.concatenate([_wrap16_rep(f) for f in flat_parts], axis=1))
        mask_list.append(np.ascontiguousarray(mask))

    # ---- permuted node features, transposed, padded rows, bf16, per core
    x_perm = np.zeros((n, f_in), np.float32)
    x_perm[perm] = x
    xT = []
    for c in range(n_cores):
        blk = np.zeros((rows_pad, f_in), np.float32)
        blk[:per_core] = x_perm[c * per_core : (c + 1) * per_core]
        xT.append(np.ascontiguousarray(blk.T).astype(bf16))

    # ---- weights: waug = [W | Wl | W@a_s | W@a_d], bias pre-summed -------
    g = lambda k: np.asarray(inputs[k], np.float32)
    rep = lambda v: np.ascontiguousarray(np.broadcast_to(v[None, :], (128, v.shape[0]))).astype(np.float32)

    def fold(Wk, Wlk, ask, adk):
        W, Wl = g(Wk), g(Wlk)
        a_s, a_d = g(ask), g(adk)
        h_, c_ = a_s.shape
        din = W.shape[0]
        vs = np.stack([W[:, i * c_ : (i + 1) * c_] @ a_s[i] for i in range(h_)], 1)
        vd = np.stack([W[:, i * c_ : (i + 1) * c_] @ a_d[i] for i in range(h_)], 1)
        return np.ascontiguousarray(np.concatenate([W, Wl, vs, vd], 1)).astype(bf16)

    waug1 = fold("W1", "Wl1", "a1s", "a1d")      # [50, 2056]
    waug2 = fold("W2", "Wl2", "a2s", "a2d")      # [1024, 2056]
    waug3 = fold("W3", "Wl3", "a3s", "a3d")      # [1024, 859]

    base = dict(
        waug1=waug1, waug2=waug2, waug3=waug3,
        bsum1=rep(g("b1") + g("bl1")),
        bsum2=rep(g("b2") + g("bl2")),
        bsum3=rep(g("b3") + g("bl3")),
    )
    in_maps = []
    for c in range(n_cores):
        m = dict(base)
        m["xT1"] = xT[c]
        m["src16"] = src16_list[c]
        m["maskneg"] = mask_list[c]
        in_maps.append(m)

    h1, c1 = np.asarray(inputs["a1s"]).shape
    h3, c3 = np.asarray(inputs["a3s"]).shape
    cfg = dict(
        n=n, f_in=f_in, n_cores=n_cores, per_core=per_core,
        T=T, rows_last=rows_last, rows_pad=rows_pad,
        K_t=K_t, K_off=[int(v) for v in K_off], K_sum=K_sum,
        h1=h1, c1=c1, d1=h1 * c1, h3=h3, c3=c3,
    )
    return in_maps, cfg, perm


# --------------------------------------------------------------------------
# bass program
# --------------------------------------------------------------------------

def _layer_dims(cfg):
    out = []
    for li in (1, 2, 3):
        if li < 3:
            h, c = cfg["h1"], cfg["c1"]
            din = cfg["f_in"] if li == 1 else cfg["d1"]
            nlin = cfg["d1"]
        else:
            h, c = cfg["h3"], cfg["c3"]
            din = cfg["d1"]
            nlin = cfg["c3"]
        hc = h * c
        nw = hc + nlin + 2 * h                 # psum cols: h | lin | es | ed
        fp8 = FP8L[li]
        psz = 1 if fp8 else 2                  # payload h dtype size
        hb = hc * psz                          # h bytes in payload
        esb = math.ceil(hb / 8) * 8            # es byte offset (8-align)
        pwb = math.ceil((esb + 4 * h) / 256) * 256   # payload bytes
        kch = math.ceil(din / 128)
        out.append(dict(li=li, din=din, kch=kch, h=h, c=c, hc=hc, nlin=nlin,
                        nw=nw, hb=hb, esf=esb // 4, pwb=pwb, fp8=fp8,
                        sc=0.125 if fp8 else 1.0))
    return out


def _build(cfg):
    import concourse.bass as bass
    import concourse.bacc as bacc
    import concourse.mybir as mybir
    import concourse.tile as tile
    from contextlib import ExitStack

    f32 = mybir.dt.float32
    bf = mybir.dt.bfloat16
    i16 = mybir.dt.int16
    i32 = mybir.dt.int32
    u8 = mybir.dt.uint8
    fp8e4 = mybir.dt.float8e4
    EXP = mybir.ActivationFunctionType.Exp
    COPY = mybir.ActivationFunctionType.Copy
    ALU = mybir.AluOpType
    AX = mybir.AxisListType.X

    n_cores = cfg["n_cores"]
    n = cfg["n"]
    T = cfg["T"]
    rows_last = cfg["rows_last"]
    per_core = cfg["per_core"]
    rows_pad = cfg["rows_pad"]
    K_t = cfg["K_t"]
    K_off = cfg["K_off"]
    K_sum = cfg["K_sum"]
    K_max = max(K_t)
    D1 = cfg["d1"]
    layers = _layer_dims(cfg)
    HMAX = max(L["h"] for L in layers)

    nc = bacc.Bacc(None, target_bir_lowering=False)

    # ---- parameters -----------------------------------------------------
    xT1 = nc.declare_dram_parameter("xT1", [cfg["f_in"], rows_pad], bf, isOutput=False)
    waug_p = {L["li"]: nc.declare_dram_parameter(f"waug{L['li']}", [L["din"], L["nw"]], bf, isOutput=False)
              for L in layers}
    bsum_p = {L["li"]: nc.declare_dram_parameter(f"bsum{L['li']}", [128, L["nlin"]], f32, isOutput=False)
              for L in layers}
    src16_p = nc.declare_dram_parameter("src16", [128, K_sum * 8], i16, isOutput=False)
    mask_p = nc.declare_dram_parameter("maskneg", [128, K_sum], f32, isOutput=False)
    out_p = nc.declare_dram_parameter("out", [per_core, cfg["c3"]], f32, isOutput=True)

    with tile.TileContext(nc, num_cores=n_cores) as tc, ExitStack() as ctx:
        # ---- dram scratch ----------------------------------------------
        dram = ctx.enter_context(tc.tile_pool(name="dram", bufs=1, space="DRAM"))
        u16 = mybir.dt.uint16
        pshard = {L["li"]: dram.tile([rows_pad, L["pwb"] // 2], u16, tag=f"pshard{L['li']}", name=f"pshard{L['li']}")
                  for L in layers}
        pfull = {L["li"]: dram.tile([n_cores * rows_pad, L["pwb"] // 2], u16, tag=f"pfull{L['li']}", name=f"pfull{L['li']}",
                                    addr_space="Shared") for L in layers}
        xrows = {li: dram.tile([rows_pad, D1], bf, tag=f"xrows{li}", name=f"xrows{li}") for li in (1, 2)}
        linb = {L["li"]: dram.tile([rows_pad, L["nlin"]], bf, tag=f"lin{L['li']}", name=f"lin{L['li']}")
                for L in layers}

        # ---- pools ------------------------------------------------------
        consts = ctx.enter_context(tc.tile_pool(name="consts", bufs=1))
        waugp = ctx.enter_context(tc.tile_pool(name="waugp", bufs=1))
        bsump = ctx.enter_context(tc.tile_pool(name="bsump", bufs=1))
        edtp = ctx.enter_context(tc.tile_pool(name="edtp", bufs=1))
        xtp = ctx.enter_context(tc.tile_pool(name="xtp", bufs=3))
        ptp = ctx.enter_context(tc.tile_pool(name="ptp", bufs=3))
        ltp = ctx.enter_context(tc.tile_pool(name="ltp", bufs=2))
        gp = ctx.enter_context(tc.tile_pool(name="gp", bufs=8))
        sgp = ctx.enter_context(tc.tile_pool(name="sgp", bufs=2))
        idxp = ctx.enter_context(tc.tile_pool(name="idxp", bufs=2))
        wep = ctx.enter_context(tc.tile_pool(name="wep", bufs=2))
        dgp = ctx.enter_context(tc.tile_pool(name="dgp", bufs=2))
        epip = ctx.enter_context(tc.tile_pool(name="epip", bufs=1))
        recp = ctx.enter_context(tc.tile_pool(name="recp", bufs=4))
        psum_d = ctx.enter_context(tc.tile_pool(name="psum_d", bufs=1, space="PSUM"))
        psum_a = ctx.enter_context(tc.tile_pool(name="psum_a", bufs=1, space="PSUM"))

        # ---- constants ---------------------------------------------------
        iota_i = consts.tile([128, 128], i32, tag="iota_i")
        nc.gpsimd.iota(iota_i[:, :], pattern=[[1, 128]], base=0, channel_multiplier=0)
        pidx_i = consts.tile([128, 1], i32, tag="pidx_i")
        nc.gpsimd.iota(pidx_i[:, :], pattern=[[1, 1]], base=0, channel_multiplier=1)
        iota_f = consts.tile([128, 128], f32, tag="iota_f")
        nc.vector.tensor_copy(iota_f[:, :], iota_i[:, :])
        pidx_f = consts.tile([128, 1], f32, tag="pidx_f")
        nc.vector.tensor_copy(pidx_f[:, :], pidx_i[:, :])
        ident = consts.tile([128, 128], bf, tag="ident")
        nc.vector.tensor_scalar(out=ident[:, :], in0=iota_f[:, :],
                                scalar1=pidx_f[:, 0:1], scalar2=None, op0=ALU.is_equal)

        def rows_of(t):
            return 128 if t < T - 1 else rows_last

        # ------------------------------------------------------------------
        for L in layers:
            li, DIN, KCH = L["li"], L["din"], L["kch"]
            H, C, HC, NLIN, NW = L["h"], L["c"], L["hc"], L["nlin"], L["nw"]
            HB, ESF, PWB, SC = L["hb"], L["esf"], L["pwb"], L["sc"]
            hdt = fp8e4 if L["fp8"] else bf
            PWE = PWB // (1 if L["fp8"] else 2)   # payload row in hdt elems

            # layer constants
            wt = [waugp.tile([128, NW], bf, tag=f"waug_kc{k}", name=f"waug_kc{k}") for k in range(KCH)]
            for k in range(KCH):
                kk = min(128, DIN - k * 128)
                nc.sync.dma_start(out=wt[k][:kk, :], in_=waug_p[li][k * 128 : k * 128 + kk, :])
            bsum = bsump.tile([128, NLIN], f32, tag="bsum")
            nc.sync.dma_start(out=bsum[:, :], in_=bsum_p[li][:, :])
            edt_all = edtp.tile([128, T * HMAX], f32, tag="edt")

            # ---------------- dense phase --------------------------------
            for t in range(T):
                r = rows_of(t)
                pd = psum_d.tile([128, NW], f32, tag="pd", name="pd")
                for k in range(KCH):
                    kk = min(128, DIN - k * 128)
                    lhsT = xtp.tile([128, 128], bf, tag="lhsT", name="lhsT")
                    if li == 1:
                        nc.sync.dma_start(out=lhsT[:kk, :], in_=xT1[:, t * 128 : (t + 1) * 128])
                    else:
                        nc.sync.dma_start(
                            out=lhsT[:, :],
                            in_=xrows[li - 1][t * 128 : (t + 1) * 128, k * 128 : (k + 1) * 128],
                            transpose=True,
                        )
                    for nb in range(math.ceil(NW / 512)):
                        w = min(512, NW - nb * 512)
                        nc.tensor.matmul(
                            pd[:, nb * 512 : nb * 512 + w],
                            lhsT[:kk, :],
                            wt[k][:kk, nb * 512 : nb * 512 + w],
                            start=(k == 0),
                            stop=(k == KCH - 1),
                        )
                # stash ed for the aggregation phase (SBUF-resident)
                nc.vector.tensor_copy(edt_all[:, t * HMAX : t * HMAX + H],
                                      pd[:, HC + NLIN + H : HC + NLIN + 2 * H])
                # payload: h (scaled, fp8/bf16) + es f32 tail
                pt = ptp.tile([128, PWB // 2], u16, tag="pt")
                if HB < ESF * 4:
                    nc.vector.memset(pt[:, HB // 2 : ESF * 2], 0.0)
                if ESF * 4 + 4 * H < PWB:
                    nc.vector.memset(pt[:, ESF * 2 + 2 * H :], 0.0)
                hview = pt.bitcast(hdt)[:, 0:HC]
                nc.vector.tensor_scalar(out=hview, in0=pd[:, 0:HC],
                                        scalar1=SC, scalar2=None, op0=ALU.mult)
                ptf = pt.bitcast(f32)
                nc.vector.tensor_copy(ptf[:, ESF : ESF + H], pd[:, HC + NLIN : HC + NLIN + H])
                nc.sync.dma_start(out=pshard[li][t * 128 : (t + 1) * 128, :], in_=pt[:, :])
                # lin + bias
                lt = ltp.tile([128, NLIN], bf, tag="lt")
                nc.vector.tensor_tensor(out=lt[:, :], in0=pd[:, HC : HC + NLIN], in1=bsum[:, :], op=ALU.add)
                nc.sync.dma_start(out=linb[li][t * 128 : (t + 1) * 128, :], in_=lt[:, :])

            # ---------------- all-gather ---------------------------------
            nc.gpsimd.collective_compute(
                "AllGather",
                ALU.bypass,
                replica_groups=[list(range(n_cores))],
                ins=[pshard[li].opt()],
                outs=[pfull[li].opt()],
            )

            # ---------------- aggregation phase --------------------------
            for t in range(T):
                r = rows_of(t)
                K = K_t[t]
                s16 = idxp.tile([128, K_max * 8], i16, tag="s16")
                msk = idxp.tile([128, K_max], f32, tag="msk")
                nc.sync.dma_start(out=s16[:, : K * 8], in_=src16_p[:, K_off[t] * 8 : (K_off[t] + K) * 8])
                nc.sync.dma_start(out=msk[:, :K], in_=mask_p[:, K_off[t] : K_off[t] + K])
                sG = sgp.tile([128, PWB // 2], u16, tag="sG")
                nc.sync.dma_start(out=sG[:, :], in_=pshard[li][t * 128 : (t + 1) * 128, :])
                GRP = 6        # chunks per gather group (768 idxs: HW limit <1024)
                NG = math.ceil(K / GRP)
                Gs = []
                for gi in range(NG):
                    kg = min(GRP, K - gi * GRP)
                    Gt = gp.tile([128, GRP * PWB // 2], u16, tag="G", name=f"G{gi}")
                    nc.gpsimd.dma_gather(
                        out_ap=Gt[:, : kg * PWB // 2].rearrange("p (k w) -> p k w", k=kg),
                        in_ap=pfull[li][:, :],
                        idxs_ap=s16[:, gi * GRP * 8 : (gi * GRP + kg) * 8],
                        num_idxs=kg * 128,
                        num_idxs_reg=kg * 128,
                        elem_size=PWB // 2,
                    )
                    Gs.append((Gt, kg))
                sGf = sG.bitcast(f32)
                sGh = sG.bitcast(hdt)
                edt = edt_all[:, t * HMAX : t * HMAX + H]

                # logits: tl = es[src] + ed[dst] (+mask), lrelu, exp
                wea = wep.tile([128, (K_max + 1) * H], f32, tag="wea")
                tl = wep.tile([128, (K_max + 1) * H], f32, tag="tl")
                for gi, (Gt, kg) in enumerate(Gs):
                    Gf = Gt[:, : kg * PWB // 2].bitcast(f32).rearrange("p (k w) -> p k w", k=kg)
                    tl3 = tl[:, gi * GRP * H : (gi * GRP + kg) * H].rearrange("p (k h) -> p k h", k=kg)
                    nc.vector.tensor_tensor(
                        out=tl3, in0=Gf[:, :, ESF : ESF + H],
                        in1=edt.rearrange("p (k h) -> p k h", k=1).broadcast_to([128, kg, H]),
                        op=ALU.add,
                    )
                nc.vector.tensor_tensor(
                    out=tl[:, : K * H].rearrange("p (k h) -> p k h", k=K),
                    in0=tl[:, : K * H].rearrange("p (k h) -> p k h", k=K),
                    in1=msk[:, :K].rearrange("p (k h) -> p k h", h=1).broadcast_to([128, K, H]),
                    op=ALU.add,
                )
                nc.vector.tensor_tensor(
                    out=tl[:, K * H : (K + 1) * H], in0=sGf[:, ESF : ESF + H],
                    in1=edt, op=ALU.add,
                )
                nc.vector.scalar_tensor_tensor(
                    out=tl[:, : (K + 1) * H], in0=tl[:, : (K + 1) * H],
                    scalar=0.2, in1=tl[:, : (K + 1) * H],
                    op0=ALU.mult, op1=ALU.max,
                )
                nc.scalar.activation(wea[:, : (K + 1) * H], tl[:, : (K + 1) * H], EXP)

                # denominator -> reciprocal (payload scale folded in)
                den = recp.tile([128, HMAX], f32, tag="den")
                nc.vector.reduce_sum(
                    den[:, :H],
                    wea[:, : (K + 1) * H].rearrange("p (k h) -> p h k", h=H),
                    axis=AX,
                )
                rec = recp.tile([128, HMAX], f32, tag="rec")
                nc.vector.tensor_scalar(out=den[:, :H], in0=den[:, :H],
                                        scalar1=SC, scalar2=None, op0=ALU.mult)
                nc.vector.reciprocal(rec[:, :H], den[:, :H])

                # weighted segment-sum via diagonal matmuls (head-outer so
                # each head's PSUM accumulation group closes before the next
                # opens in the same bank)
                CP = math.ceil(C / 128) * 128     # per-head psum col stride
                ps = psum_a.tile([128, H * CP], f32, tag="ps", name="ps")
                wea3 = wea[:, : (K + 1) * H].rearrange("p (k h) -> p k h", h=H)
                n_act = 0        # heads whose diag builds go to ACT
                for h in range(H):
                    dgh = dgp.tile([128, (K_max + 1) * 128], bf, tag="dgh")
                    if h < n_act:
                        for j in range(K + 1):
                            nc.scalar.activation(
                                dgh[:, j * 128 : (j + 1) * 128], ident[:, :], COPY,
                                scale=wea[:, j * H + h : j * H + h + 1],
                            )
                    else:
                        nc.vector.tensor_tensor(
                            out=dgh[:, : (K + 1) * 128].rearrange("p (k q) -> p k q", k=K + 1),
                            in0=ident.rearrange("p (k q) -> p k q", k=1).broadcast_to([128, K + 1, 128]),
                            in1=wea3[:, :, h : h + 1].broadcast_to([128, K + 1, 128]),
                            op=ALU.mult,
                        )
                    for j in range(K + 1):        # j==K is the self chunk
                        if j == K:
                            rhs = sGh[:, h * C : (h + 1) * C]
                        else:
                            Gt, _ = Gs[j // GRP]
                            jj = j % GRP
                            rhs = Gt.bitcast(hdt)[:, jj * PWE + h * C : jj * PWE + (h + 1) * C]
                        nc.tensor.matmul(
                            ps[:, h * CP : h * CP + C],
                            dgh[:, j * 128 : (j + 1) * 128],
                            rhs,
                            start=(j == 0),
                            stop=(j == K),
                        )

                # epilogue: drain PSUM to SBUF with one idle-ACT copy so the
                # next tile's matmuls can reuse the psum slot immediately
                xtr = epip.tile([128, H * CP], f32, tag="s")
                nc.scalar.activation(xtr[:, :], ps[:, :], COPY)
                xt = epip.tile([128, HC], f32, tag="xt")
                for h in range(H):
                    nc.vector.tensor_scalar(
                        out=xt[:, h * C : (h + 1) * C], in0=xtr[:, h * CP : h * CP + C],
                        scalar1=rec[:, h : h + 1], scalar2=None, op0=ALU.mult,
                    )
                lt2 = ltp.tile([128, NLIN], bf, tag="lt2")
                nc.sync.dma_start(out=lt2[:, :], in_=linb[li][t * 128 : (t + 1) * 128, :])
                if li < 3:
                    s = epip.tile([128, HC], f32, tag="s")
                    u = epip.tile([128, HC], f32, tag="u")
                    e = epip.tile([128, HC], f32, tag="e")
                    xo = epip.tile([128, HC], bf, tag="xo")
                    nc.vector.tensor_tensor(out=s[:, :], in0=xt[:, :], in1=lt2[:, :], op=ALU.add)
                    nc.vector.tensor_scalar(out=u[:, :], in0=s[:, :], scalar1=0.0, scalar2=None, op0=ALU.min)
                    nc.scalar.activation(e[:, :], u[:, :], EXP)
                    # elu(s) = relu(s) + exp(min(s,0)) - 1
                    nc.vector.scalar_tensor_tensor(
                        out=s[:, :], in0=s[:, :], scalar=0.0, in1=e[:, :],
                        op0=ALU.max, op1=ALU.add,
                    )
                    nc.vector.tensor_scalar(out=xo[:, :], in0=s[:, :],
                                            scalar1=-1.0, scalar2=None, op0=ALU.add)
                    nc.sync.dma_start(out=xrows[li][t * 128 : (t + 1) * 128, :], in_=xo[:, :])
                else:
                    xt3 = xt.rearrange("p (h c) -> p h c", h=H)
                    m1 = epip.tile([128, 3, C], f32, tag="m1")
                    nc.vector.tensor_tensor(out=m1[:, :, :], in0=xt3[:, 0:3, :], in1=xt3[:, 3:6, :], op=ALU.add)
                    m2 = epip.tile([128, C], f32, tag="m2")
                    nc.vector.tensor_tensor(out=m2[:, :], in0=m1[:, 0, :], in1=m1[:, 1, :], op=ALU.add)
                    nc.vector.tensor_tensor(out=m2[:, :], in0=m2[:, :], in1=m1[:, 2, :], op=ALU.add)
                    ot = epip.tile([128, C], f32, tag="ot")
                    nc.vector.scalar_tensor_tensor(
                        out=ot[:r, :], in0=m2[:r, :], scalar=1.0 / H, in1=lt2[:r, :],
                        op0=ALU.mult, op1=ALU.add,
                    )
                    nc.sync.dma_start(out=out_p[t * 128 : t * 128 + r, :], in_=ot[:r, :])

    nc.finalize()
    return nc


# --------------------------------------------------------------------------
# runner
# --------------------------------------------------------------------------

def _run(inputs, sim=False, trace=False, n_cores=N_CORES, tmpdir=None):
    in_maps, cfg, perm = _host_prep(inputs, n_cores)
    nc = _build(cfg)
    if sim:
        import concourse.bass_interp as bass_interp

        msim = bass_interp.MultiCoreSim(nc, n_cores)
        for c in range(n_cores):
            for k, v in in_maps[c].items():
                msim.cores[c].tensor(k)[:] = v
        msim.simulate(check_with_hw=False)
        outs = [np.array(msim.cores[c].mem_tensor("out")) for c in range(n_cores)]
        exec_ns = None
    else:
        from concourse.bass_utils import run_bass_kernel_spmd

        res = run_bass_kernel_spmd(
            nc, in_maps, list(range(n_cores)), trace=trace, tmpdir=tmpdir
        )
        outs = [res.results[c]["out"] for c in range(n_cores)]
        exec_ns = res.exec_time_ns
    out_new = np.concatenate(outs, 0)
    out = np.empty_like(out_new)
    out[...] = out_new[perm]
    return out.astype(np.float32), exec_ns


def kernel(**inputs) -> np.ndarray:
    out, _ = _run(inputs)
    return out
